# revision 1
# baseline (speedup 1.0000x reference)
"""Cluster-local attention Trainium2 kernel.

Reference semantics (see problem):
    order = argsort(cluster_label, stable); xs = x[:, order]
    qkv = xs @ W_qkv + b_qkv ; q,k,v split, 8 heads x 64
    per (head, window of 256 sorted tokens): softmax(q k^T / 8) @ v
    h = attn_out @ W_out + b_out + xs            (returned in sorted order)

Sharding: 64 windows of 256 tokens -> 8 windows (2048 tokens) per core.
Fully embarrassingly parallel (weights replicated, no collectives).

Per-core device program (all matmuls bf16, accumulation fp32 in PSUM):
  1. load x fp32 (residual) and host-pretransposed X^T bf16 [512, 2048]
  2. Q^T,K^T via W-stationary qkv^T-form matmuls, kept [dim, tok];
     V via X^T-stationary token-major matmuls, copied into per-head
     80-col slots with a ones column at +64 for the softmax row-sum
  3. per (window, head):  s^T[k, q] = (K^T)^T Q^T ; exp on ACT (scale=1/8);
     PV matmul with the ones column producing [q, 64 out | rowsum];
     normalize via reciprocal + tensor_scalar_mul -> ao [tok, 512] bf16
  4. DMA-transpose ao -> ao^T; h = ao^T-form matmul + residual x (fp32)
"""

import sys

if "/opt/trn_rl_repo" not in sys.path:
    sys.path.insert(0, "/opt/trn_rl_repo")

import numpy as np
import ml_dtypes

B = 1
L = 16384
HID = 512
NH = 8
D = 64
WIN = 256
N_CORES = 8
T = L // N_CORES            # 2048 tokens per core
TT = T // 128               # 16 token tiles per core
NWIN = T // WIN             # 8 windows per core
VSLOT = 80                  # per-head column slot in V65 (64 v + 1 ones + pad)

_PROGRAM_CACHE = {}


def _build_program(has_bq: bool, has_bo: bool):
    import concourse.bacc as bacc
    import concourse.tile as tile
    import concourse.mybir as mybir

    fp32 = mybir.dt.float32
    bf16 = mybir.dt.bfloat16
    AF = mybir.ActivationFunctionType

    nc = bacc.Bacc("TRN2", target_bir_lowering=False, debug=False,
                   enable_asserts=False, num_devices=N_CORES)

    x_d = nc.dram_tensor("x", [T, HID], fp32, kind="ExternalInput").ap()
    xt_d = nc.dram_tensor("xt", [HID, T], bf16, kind="ExternalInput").ap()
    wqkv_d = nc.dram_tensor("wqkv", [HID, 3 * HID], bf16, kind="ExternalInput").ap()
    wout_d = nc.dram_tensor("wout", [HID, HID], bf16, kind="ExternalInput").ap()
    if has_bq:
        bqkv_d = nc.dram_tensor("bqkv", [3 * HID], fp32, kind="ExternalInput").ap()
    if has_bo:
        bout_d = nc.dram_tensor("bout", [HID], fp32, kind="ExternalInput").ap()
    h_d = nc.dram_tensor("h", [T, HID], fp32, kind="ExternalOutput").ap()

    import os
    debug = bool(os.environ.get("CLA_DEBUG"))
    stages = int(os.environ.get("CLA_STAGES", "4"))
    if debug:
        qkt_d = nc.dram_tensor("dbg_qkt", [128, 8 * T], bf16,
                               kind="ExternalOutput").ap()
        v65_d = nc.dram_tensor("dbg_v65", [128, TT * NH * 65], bf16,
                               kind="ExternalOutput").ap()
        ao_d = nc.dram_tensor("dbg_ao", [128, TT * HID], bf16,
                              kind="ExternalOutput").ap()
        ex_d = nc.dram_tensor("dbg_ex", [128, 2 * WIN], bf16,
                              kind="ExternalOutput").ap()

    from contextlib import ExitStack

    with tile.TileContext(nc) as tc, ExitStack() as ctx:
        consts = ctx.enter_context(tc.tile_pool(name="consts", bufs=1))
        vt_p = ctx.enter_context(tc.tile_pool(name="vt", bufs=4))
        exp_p = ctx.enter_context(tc.tile_pool(name="expp", bufs=4))
        h_p = ctx.enter_context(tc.tile_pool(name="hp", bufs=4))
        rc_p = ctx.enter_context(tc.tile_pool(name="rcp", bufs=8))
        import os
        _aot_pe = os.environ.get("CLA_AOT", "pe") == "pe"
        proj_ps = ctx.enter_context(tc.tile_pool(
            name="proj_ps", bufs=int(os.environ.get("CLA_PJB", "2")), space="PSUM"))
        st_ps = ctx.enter_context(tc.tile_pool(
            name="st_ps", bufs=int(os.environ.get("CLA_STB", "2")), space="PSUM"))
        pv_ps = ctx.enter_context(tc.tile_pool(
            name="pv_ps",
            bufs=int(os.environ.get("CLA_PVB", "2" if _aot_pe else "4")),
            space="PSUM"))
        if _aot_pe:
            tp_ps = ctx.enter_context(tc.tile_pool(
                name="tp_ps", bufs=2, space="PSUM"))


        # ---- persistent SBUF tensors -------------------------------------
        xf = consts.tile([128, TT * HID], fp32)           # x fp32, token tiles
        wqkv = consts.tile([128, 4 * 3 * HID], bf16)      # 4 hidden chunks
        wout = consts.tile([128, 4 * HID], bf16)
        xt = consts.tile([128, 4 * T], bf16)              # X^T, 4 hidden chunks
        qkt = consts.tile([128, 8 * T], bf16)             # Q^T|K^T, 8 dim chunks
        v65 = consts.tile([128, TT * NH * VSLOT], bf16)   # token-major V+ones
        ao = consts.tile([128, TT * HID], bf16)           # attn out token-major
        aot = consts.tile([128, 4 * T], bf16)             # ao^T, 4 chunks

        # CLA_REPEAT > 1 re-emits the whole body for dispatch-overhead-free
        # timing (wall(R) - wall(1) = (R-1) * kernel_time).
        repeat = int(os.environ.get("CLA_REPEAT", "1"))
        for _rep in range(repeat):
            _emit_body(nc, tc, mybir, locals())

    nc.compile()
    return nc


def _emit_body(nc, tc, mybir, env):
    import os
    fp32 = mybir.dt.float32
    bf16 = mybir.dt.bfloat16
    AF = mybir.ActivationFunctionType
    (has_bq, has_bo, debug, stages) = (env["has_bq"], env["has_bo"],
                                       env["debug"], env["stages"])
    (consts, vt_p, exp_p, h_p, rc_p) = (env["consts"], env["vt_p"],
                                        env["exp_p"], env["h_p"], env["rc_p"])
    (proj_ps, st_ps, pv_ps) = (env["proj_ps"], env["st_ps"], env["pv_ps"])
    tp_ps = env.get("tp_ps")
    (xf, wqkv, wout, xt, qkt, v65, ao, aot) = (
        env["xf"], env["wqkv"], env["wout"], env["xt"], env["qkt"],
        env["v65"], env["ao"], env["aot"])
    (x_d, xt_d, wqkv_d, wout_d, h_d) = (env["x_d"], env["xt_d"],
                                        env["wqkv_d"], env["wout_d"],
                                        env["h_d"])
    if has_bq:
        bqkv_d = env["bqkv_d"]
    if has_bo:
        bout_d = env["bout_d"]
    if debug:
        (qkt_d, v65_d, ao_d, ex_d) = (env["qkt_d"], env["v65_d"],
                                      env["ao_d"], env["ex_d"])

    if True:
        # ---- loads -------------------------------------------------------
        nc.sync.dma_start(
            out=xf.rearrange("p (t d) -> p t d", d=HID),
            in_=x_d.rearrange("(t p) d -> p t d", p=128))
        for c in range(4):
            nc.sync.dma_start(
                out=xt[:, c * T:(c + 1) * T],
                in_=xt_d[c * 128:(c + 1) * 128, :])
            nc.sync.dma_start(
                out=wqkv[:, c * 3 * HID:(c + 1) * 3 * HID],
                in_=wqkv_d[c * 128:(c + 1) * 128, :])
            nc.sync.dma_start(
                out=wout[:, c * HID:(c + 1) * HID],
                in_=wout_d[c * 128:(c + 1) * 128, :])

        if has_bq or has_bo:
            ones_row = consts.tile([1, 128], bf16)
            nc.vector.memset(ones_row, 1.0)
        if has_bq:
            bq_cols = consts.tile([128, 12], fp32)
            nc.sync.dma_start(out=bq_cols,
                              in_=bqkv_d.rearrange("(m p) -> p m", p=128))
            bqv_f = consts.tile([1, HID], fp32)
            nc.sync.dma_start(out=bqv_f,
                              in_=bqkv_d[2 * HID:3 * HID].rearrange(
                                  "(o d) -> o d", o=1))
            bqv_row = consts.tile([1, HID], bf16)
            nc.vector.tensor_copy(bqv_row, bqv_f)
        if has_bo:
            bout_f = consts.tile([1, HID], fp32)
            nc.sync.dma_start(out=bout_f,
                              in_=bout_d.rearrange("(o d) -> o d", o=1))
            bout_bf = consts.tile([1, HID], bf16)
            nc.vector.tensor_copy(bout_bf, bout_f)
            bps = proj_ps.tile([128, HID], fp32, tag="ps")
            nc.tensor.matmul(bps, ones_row, bout_bf, start=True, stop=True)
            bbc = consts.tile([128, HID], fp32)
            nc.vector.tensor_copy(bbc, bps)

        # ones columns of V65 (col 64 of every 80-col head slot)
        nc.vector.memset(
            v65.rearrange("p (s c) -> p s c", c=VSLOT)[:, :, 64:65], 1.0)

        # ---- Q^T / K^T projections (qkv^T form, W stationary) -------------
        copy_flip = 0
        for m in range(8):
            for n in range(4):  # token chunks of 512
                ps = proj_ps.tile([128, HID], fp32, tag="ps")
                for kk in range(4):
                    nc.tensor.matmul(
                        ps,
                        wqkv[:, kk * 3 * HID + m * 128: kk * 3 * HID + (m + 1) * 128],
                        xt[:, kk * T + n * 512: kk * T + (n + 1) * 512],
                        start=(kk == 0), stop=(kk == 3))
                dst = qkt[:, m * T + n * 512: m * T + (n + 1) * 512]
                if copy_flip == 0:
                    if has_bq:
                        nc.vector.tensor_scalar_add(dst, ps, bq_cols[:, m:m + 1])
                    else:
                        nc.vector.tensor_copy(dst, ps)
                else:
                    if has_bq:
                        nc.scalar.activation(dst, ps, AF.Identity,
                                             bias=bq_cols[:, m:m + 1])
                    else:
                        nc.scalar.activation(dst, ps, AF.Copy)
                copy_flip ^= 1

        # ---- V projection, token-major (X^T stationary) -------------------
        for t in range(TT):
            ps = proj_ps.tile([128, HID], fp32, tag="ps")
            if has_bq:
                nc.tensor.matmul(ps, ones_row, bqv_row,
                                 start=True, stop=False)
            for kk in range(4):
                nc.tensor.matmul(
                    ps,
                    xt[:, kk * T + t * 128: kk * T + (t + 1) * 128],
                    wqkv[:, kk * 3 * HID + 2 * HID: kk * 3 * HID + 3 * HID],
                    start=(kk == 0 and not has_bq), stop=(kk == 3))
            vt = vt_p.tile([128, HID], bf16)
            nc.vector.tensor_copy(vt, ps)
            # one strided copy scatters all 8 head slices into their slots
            nc.gpsimd.tensor_copy(
                v65[:, t * NH * VSLOT: (t + 1) * NH * VSLOT]
                .rearrange("p (h c) -> p h c", c=VSLOT)[:, :, 0:64],
                vt.rearrange("p (h c) -> p h c", c=64))

        # ---- attention ----------------------------------------------------
        # heads_per_exp (1 or 2) trades ACT overhead against PSUM pressure
        hpe = int(os.environ.get("CLA_HPE", "1"))
        for w in range(NWIN if stages >= 2 else 0):
            for hp in range(NH // hpe):
                st = st_ps.tile([128, hpe * 2 * WIN], fp32)
                for hi in range(hpe):
                    hh = hpe * hp + hi
                    mq = hh // 2
                    mk = 4 + hh // 2
                    prow = (hh % 2) * 64
                    for kc in range(2):
                        nc.tensor.matmul(
                            st[:, hi * 2 * WIN + kc * WIN:
                               hi * 2 * WIN + (kc + 1) * WIN],
                            qkt[prow:prow + 64,
                                mk * T + w * WIN + kc * 128:
                                mk * T + w * WIN + (kc + 1) * 128],
                            qkt[prow:prow + 64,
                                mq * T + w * WIN: mq * T + (w + 1) * WIN],
                            start=True, stop=True)
                ex = exp_p.tile([128, hpe * 2 * WIN], bf16)
                nc.scalar.activation(ex, st, AF.Exp, scale=1.0 / np.sqrt(D))
                if debug and w == 0 and hp == 0:
                    nc.sync.dma_start(out=ex_d, in_=ex[:, 0:2 * WIN])
                for hi in range(hpe):
                    hh = hpe * hp + hi
                    # both q-chunks of one head share a 1-bank psum tile:
                    # qc0 at cols 0:65, qc1 at 65:130 -> one reciprocal op
                    pv = pv_ps.tile([128, 130], fp32)
                    for qc in range(2):
                        for kc in range(2):
                            nc.tensor.matmul(
                                pv[:, qc * 65: qc * 65 + 65],
                                ex[:, hi * 2 * WIN + kc * WIN + qc * 128:
                                   hi * 2 * WIN + kc * WIN + (qc + 1) * 128],
                                v65[:, (2 * w + kc) * NH * VSLOT + hh * VSLOT:
                                    (2 * w + kc) * NH * VSLOT + hh * VSLOT + 65],
                                start=(kc == 0), stop=(kc == 1))
                    rc = rc_p.tile([128, 2], fp32)
                    nc.vector.reciprocal(
                        rc, pv.rearrange("p (q c) -> p q c", c=65)[:, :, 64])
                    for qc in range(2):
                        t = 2 * w + qc
                        nc.vector.tensor_scalar_mul(
                            ao[:, t * HID + hh * 64: t * HID + (hh + 1) * 64],
                            pv[:, qc * 65: qc * 65 + 64],
                            rc[:, qc:qc + 1])

        if debug:
            nc.sync.dma_start(out=qkt_d, in_=qkt)
            nc.sync.dma_start(
                out=v65_d.rearrange("p (s c) -> p s c", c=65),
                in_=v65.rearrange("p (s c) -> p s c", c=VSLOT)[:, :, 0:65])
            nc.sync.dma_start(out=ao_d, in_=ao)

        # ---- ao^T ---------------------------------------------------------
        aot_mode = os.environ.get("CLA_AOT", "pe")
        if aot_mode == "pe" and stages >= 3:
            from concourse.masks import make_identity
            ident = consts.tile([128, 128], bf16)
            make_identity(nc, ident)
            tp_flip = 0
            # two PE transposes (chunks c, c+1 of one token tile) share a
            # psum tile; one copy moves both to SBUF.  aot chunks c and c+1
            # sit T apart, so the copy uses a strided 3D dest AP.
            for t in range(TT):
                for c2 in range(2):
                    tp = tp_ps.tile([128, 256], bf16)
                    for j in range(2):
                        c = 2 * c2 + j
                        nc.tensor.transpose(
                            tp[:, j * 128:(j + 1) * 128],
                            ao[:, t * HID + c * 128: t * HID + (c + 1) * 128],
                            ident)
                    dst = aot.rearrange("p (c x) -> p c x", x=T)[
                        :, 2 * c2: 2 * c2 + 2, t * 128:(t + 1) * 128]
                    if tp_flip == 0:
                        nc.vector.tensor_copy(
                            dst, tp.rearrange("p (j x) -> p j x", x=128))
                    else:
                        nc.scalar.activation(
                            dst, tp.rearrange("p (j x) -> p j x", x=128),
                            AF.Copy)
                    tp_flip ^= 1
        else:
            for t in range(TT if stages >= 3 else 0):
                for c in range(4):
                    nc.sync.dma_start(
                        out=aot[:, c * T + t * 128: c * T + (t + 1) * 128],
                        in_=ao[:, t * HID + c * 128: t * HID + (c + 1) * 128],
                        transpose=True)

        # ---- output projection + residual ---------------------------------
        for t in range(TT if stages >= 4 else 0):
            ps = proj_ps.tile([128, HID], fp32, tag="ps")
            for kk in range(4):
                nc.tensor.matmul(
                    ps,
                    aot[:, kk * T + t * 128: kk * T + (t + 1) * 128],
                    wout[:, kk * HID:(kk + 1) * HID],
                    start=(kk == 0), stop=(kk == 3))
            ht = h_p.tile([128, HID], fp32)
            nc.vector.tensor_add(ht, ps, xf[:, t * HID:(t + 1) * HID])
            if has_bo:
                nc.vector.tensor_add(ht, ht, bbc)
            nc.gpsimd.dma_start(out=h_d[t * 128:(t + 1) * 128, :], in_=ht)

    nc.compile()
    return nc


def _get_program(has_bq: bool, has_bo: bool):
    key = (has_bq, has_bo)
    if key not in _PROGRAM_CACHE:
        _PROGRAM_CACHE[key] = _build_program(has_bq, has_bo)
    return _PROGRAM_CACHE[key]


def make_in_maps(x, cluster_label, W_qkv, b_qkv, W_out, b_out):
    """Host-side prep: sort tokens, shard, cast weights. Returns
    (in_maps, has_bq, has_bo)."""
    x = np.asarray(x, dtype=np.float32).reshape(L, HID)
    labels = np.asarray(cluster_label)
    order = np.argsort(labels, kind="stable")
    if not np.array_equal(order, np.arange(L)):
        xs = np.ascontiguousarray(x[order])
    else:
        xs = np.ascontiguousarray(x)
    wqkv_bf = np.asarray(W_qkv, dtype=np.float32).astype(ml_dtypes.bfloat16)
    wout_bf = np.asarray(W_out, dtype=np.float32).astype(ml_dtypes.bfloat16)
    bq = np.asarray(b_qkv, dtype=np.float32).reshape(3 * HID)
    bo = np.asarray(b_out, dtype=np.float32).reshape(HID)
    has_bq = bool(np.any(bq != 0))
    has_bo = bool(np.any(bo != 0))

    xs_bf = xs.astype(ml_dtypes.bfloat16)
    in_maps = []
    for c in range(N_CORES):
        m = {
            "x": xs[c * T:(c + 1) * T],
            "xt": np.ascontiguousarray(xs_bf[c * T:(c + 1) * T].T),
            "wqkv": wqkv_bf,
            "wout": wout_bf,
        }
        if has_bq:
            m["bqkv"] = bq
        if has_bo:
            m["bout"] = bo
        in_maps.append(m)
    return in_maps, has_bq, has_bo


def kernel(x, cluster_label, W_qkv, b_qkv, W_out, b_out):
    from concourse.bass_utils import run_bass_kernel_spmd

    in_maps, has_bq, has_bo = make_in_maps(
        x, cluster_label, W_qkv, b_qkv, W_out, b_out)
    nc = _get_program(has_bq, has_bo)
    res = run_bass_kernel_spmd(nc, in_maps, list(range(N_CORES)), trace=False)
    h = np.concatenate([res.results[c]["h"] for c in range(N_CORES)], axis=0)
    return h.reshape(B, L, HID).astype(np.float32)

